# revision 4
# baseline (speedup 1.0000x reference)
"""Variable-length average pooling (prefix mean over seq axis) on 8 trn2 cores.

Strategy (pure data parallelism over batch):
  - eff_len[b] = lengths[b] if >0 else L.  pooled[b] = sum_{l<eff} x[b,l,:] / eff.
  - Sort batches by eff_len desc, snake-assign 16 per core so per-core work and
    per-slot length profiles are balanced across cores (~0.8% imbalance).
  - One SPMD Bass program shared by all 8 cores: slot j processes
    ceil(max_core_len_j/128) L-chunks of [rows<=128, 2048]; rows beyond a
    core's own length are zeroed by the per-core mask weights, so only the
    slot-max structure is baked into the program (+5% extra DMA vs ideal).
  - Reduction over L is a matmul: psum[1, 512] += maskcol[rows,1].T @ tile[rows,512]
    with maskcol[p] = (128k+p < eff)/eff  (scale folded in).
  - PSUM [1,2048] -> SBUF via ScalarE copy -> DMA out.
"""

import os

import numpy as np

import concourse.bacc as bacc
import concourse.mybir as mybir
from concourse.tile import TileContext
from concourse.bass_utils import run_bass_kernel_spmd

B, L, D = 128, 1024, 2048
NCORES = 8
SLOTS = B // NCORES  # 16
PCHUNK = 128         # L-rows per chunk (partition dim of the tile)
MAXK = L // PCHUNK   # 8
NTILE = 512          # matmul moving free dim (one PSUM bank of fp32)

# fp32 moving operand runs at 1/4 PE rate; float32r runs at full rate for
# N>=256. Set MM_DTYPE env to experiment; default chosen by measurement.
_MM_DT = {
    "f32": mybir.dt.float32,
    "f32r": mybir.dt.float32r,
}[os.environ.get("MM_DTYPE", "f32")]

LAST_RESULTS = None  # BassKernelResults of the most recent device run


def _plan(eff):
    """Snake-assign sorted batches to cores; return (perm[core][slot], chunk rows)."""
    order = np.argsort(-eff, kind="stable")
    cores = [[] for _ in range(NCORES)]
    for i, idx in enumerate(order):
        blk, pos = divmod(i, NCORES)
        c = pos if blk % 2 == 0 else NCORES - 1 - pos
        cores[c].append(int(idx))
    slot_max = [max(eff[cores[c][s]] for c in range(NCORES)) for s in range(SLOTS)]
    slot_rows = []
    for s in range(SLOTS):
        m = int(slot_max[s])
        nk = -(-m // PCHUNK)
        slot_rows.append(tuple(min(PCHUNK, m - PCHUNK * k) for k in range(nk)))
    return cores, tuple(slot_rows)


_PROGRAM_CACHE = {}


def _build_program(slot_rows):
    # Bacc (not raw Bass): its compile pass splits multi-sem waits and moves
    # matmul waits onto ldweights — walrus allows only 1 wait per instruction.
    nc = bacc.Bacc(None, target_bir_lowering=False)
    f32 = mybir.dt.float32
    feat = nc.dram_tensor("features", [SLOTS, L, D], f32, kind="ExternalInput")
    maskt = nc.dram_tensor("maskt", [PCHUNK, SLOTS * MAXK], f32, kind="ExternalInput")
    out = nc.dram_tensor("out", [SLOTS, D], f32, kind="ExternalOutput")

    with TileContext(nc) as tc:
        with (
            tc.tile_pool(name="mask", bufs=1) as mpool,
            tc.tile_pool(name="tiles", bufs=4) as tpool,
            tc.tile_pool(name="psum", bufs=2, space="PSUM") as ppool,
            tc.tile_pool(name="outs", bufs=3) as opool,
        ):
            mask_tile = mpool.tile([PCHUNK, SLOTS * MAXK], f32)
            nc.sync.dma_start(out=mask_tile[:], in_=maskt[:])
            for s in range(SLOTS):
                rows_list = slot_rows[s]
                nk = len(rows_list)
                psum_t = ppool.tile([1, D], f32)
                for k, rows in enumerate(rows_list):
                    tile = tpool.tile([PCHUNK, D], f32)
                    nc.sync.dma_start(
                        out=tile[:rows], in_=feat[s, k * PCHUNK : k * PCHUNK + rows, :]
                    )
                    col = s * MAXK + k
                    for j in range(D // NTILE):
                        nc.tensor.matmul(
                            psum_t[0:1, j * NTILE : (j + 1) * NTILE],
                            mask_tile[0:rows, col : col + 1].bitcast(_MM_DT),
                            tile[0:rows, j * NTILE : (j + 1) * NTILE].bitcast(_MM_DT),
                            start=(k == 0),
                            stop=(k == nk - 1),
                        )
                out_t = opool.tile([1, D], f32)
                nc.scalar.copy(out=out_t[:], in_=psum_t[:])
                nc.sync.dma_start(out=out[s : s + 1, :], in_=out_t[:])
    nc.finalize()
    return nc


def kernel(features, lengths):
    global LAST_RESULTS
    features = np.ascontiguousarray(features, dtype=np.float32)
    lengths = np.ascontiguousarray(lengths, dtype=np.int32)
    eff = np.where(lengths > 0, lengths, L).astype(np.int64)

    cores, slot_rows = _plan(eff)
    if slot_rows not in _PROGRAM_CACHE:
        _PROGRAM_CACHE[slot_rows] = _build_program(slot_rows)
    nc = _PROGRAM_CACHE[slot_rows]

    in_maps = []
    for c in range(NCORES):
        perm = cores[c]
        maskt = np.zeros((PCHUNK, SLOTS * MAXK), dtype=np.float32)
        for s, b in enumerate(perm):
            e = int(eff[b])
            for k, rows in enumerate(slot_rows[s]):
                lo = k * PCHUNK
                n_valid = min(max(e - lo, 0), PCHUNK)
                if n_valid > 0:
                    maskt[:n_valid, s * MAXK + k] = 1.0 / e
        in_maps.append({"features": features[perm], "maskt": maskt})

    trace = os.environ.get("KERNEL_TRACE", "0") == "1"
    LAST_RESULTS = run_bass_kernel_spmd(
        nc,
        in_maps,
        core_ids=list(range(NCORES)),
        trace=trace,
        trace_cores=[0] if trace else None,
    )

    out = np.empty((B, D), dtype=np.float32)
    for c in range(NCORES):
        out[np.asarray(cores[c])] = LAST_RESULTS.results[c]["out"]
    return out


# revision 5
# speedup vs baseline: 1.2036x; 1.2036x over previous
"""Variable-length average pooling (prefix mean over seq axis) on 8 trn2 cores.

Strategy (pure data parallelism over batch):
  - eff_len[b] = lengths[b] if >0 else L.  pooled[b] = sum_{l<eff} x[b,l,:] / eff.
  - Sort batches by eff_len desc, snake-assign 16 per core so per-core work and
    per-slot length profiles are balanced across cores (~0.8% imbalance).
  - One SPMD Bass program shared by all 8 cores: slot j processes
    ceil(max_core_len_j/128) L-chunks of [rows<=128, 2048]; rows beyond a
    core's own length are zeroed by the per-core mask weights, so only the
    slot-max structure is baked into the program (+5% extra DMA vs ideal).
  - Reduction over L is a matmul: psum[1, 512] += maskcol[rows,1].T @ tile[rows,512]
    with maskcol[p] = (128k+p < eff)/eff  (scale folded in).
  - PSUM [1,2048] -> SBUF via ScalarE copy -> DMA out.
"""

import os

import numpy as np

import concourse.bacc as bacc
import concourse.mybir as mybir
from concourse.tile import TileContext
from concourse.bass_utils import run_bass_kernel_spmd

B, L, D = 128, 1024, 2048
NCORES = 8
SLOTS = B // NCORES  # 16
PCHUNK = 128         # L-rows per chunk (partition dim of the tile)
MAXK = L // PCHUNK   # 8
NTILE = 512          # matmul moving free dim (one PSUM bank of fp32)

# fp32 moving operand runs at 1/4 PE rate; float32r runs at full rate for
# N>=256. Set MM_DTYPE env to experiment; default chosen by measurement.
_MM_DT = {
    "f32": mybir.dt.float32,
    "f32r": mybir.dt.float32r,
}[os.environ.get("MM_DTYPE", "f32")]

LAST_RESULTS = None  # BassKernelResults of the most recent device run


def _plan(eff):
    """Snake-assign sorted batches to cores; return (perm[core][slot], chunk rows)."""
    order = np.argsort(-eff, kind="stable")
    cores = [[] for _ in range(NCORES)]
    for i, idx in enumerate(order):
        blk, pos = divmod(i, NCORES)
        c = pos if blk % 2 == 0 else NCORES - 1 - pos
        cores[c].append(int(idx))
    slot_max = [max(eff[cores[c][s]] for c in range(NCORES)) for s in range(SLOTS)]
    slot_rows = []
    for s in range(SLOTS):
        m = int(slot_max[s])
        nk = -(-m // PCHUNK)
        slot_rows.append(tuple(min(PCHUNK, m - PCHUNK * k) for k in range(nk)))
    return cores, tuple(slot_rows)


_PROGRAM_CACHE = {}


def _build_program(slot_rows):
    # Bacc (not raw Bass): its compile pass splits multi-sem waits and moves
    # matmul waits onto ldweights — walrus allows only 1 wait per instruction.
    nc = bacc.Bacc(None, target_bir_lowering=False)
    f32 = mybir.dt.float32
    feat = nc.dram_tensor("features", [SLOTS, L, D], f32, kind="ExternalInput")
    maskt = nc.dram_tensor("maskt", [PCHUNK, SLOTS * MAXK], f32, kind="ExternalInput")
    out = nc.dram_tensor("out", [SLOTS, D], f32, kind="ExternalOutput")

    with TileContext(nc) as tc:
        with (
            tc.tile_pool(name="mask", bufs=1) as mpool,
            tc.tile_pool(name="tiles", bufs=4) as tpool,
            tc.tile_pool(name="psum", bufs=2, space="PSUM") as ppool,
            tc.tile_pool(name="outs", bufs=3) as opool,
        ):
            # float32r operands must be *produced* as float32r (BIR verifier);
            # a casting SWDGE DMA does the rounding inline.
            mm_dt = _MM_DT
            dma_in = nc.sync if mm_dt == f32 else nc.gpsimd
            mask_tile = mpool.tile([PCHUNK, SLOTS * MAXK], mm_dt)
            dma_in.dma_start(out=mask_tile[:], in_=maskt[:])
            for s in range(SLOTS):
                rows_list = slot_rows[s]
                nk = len(rows_list)
                psum_t = ppool.tile([1, D], f32)
                for k, rows in enumerate(rows_list):
                    tile = tpool.tile([PCHUNK, D], mm_dt)
                    dma_in.dma_start(
                        out=tile[:rows], in_=feat[s, k * PCHUNK : k * PCHUNK + rows, :]
                    )
                    col = s * MAXK + k
                    for j in range(D // NTILE):
                        nc.tensor.matmul(
                            psum_t[0:1, j * NTILE : (j + 1) * NTILE],
                            mask_tile[0:rows, col : col + 1],
                            tile[0:rows, j * NTILE : (j + 1) * NTILE],
                            start=(k == 0),
                            stop=(k == nk - 1),
                        )
                out_t = opool.tile([1, D], f32)
                nc.scalar.copy(out=out_t[:], in_=psum_t[:])
                nc.sync.dma_start(out=out[s : s + 1, :], in_=out_t[:])
    nc.finalize()
    return nc


def kernel(features, lengths):
    global LAST_RESULTS
    features = np.ascontiguousarray(features, dtype=np.float32)
    lengths = np.ascontiguousarray(lengths, dtype=np.int32)
    eff = np.where(lengths > 0, lengths, L).astype(np.int64)

    cores, slot_rows = _plan(eff)
    if slot_rows not in _PROGRAM_CACHE:
        _PROGRAM_CACHE[slot_rows] = _build_program(slot_rows)
    nc = _PROGRAM_CACHE[slot_rows]

    in_maps = []
    for c in range(NCORES):
        perm = cores[c]
        maskt = np.zeros((PCHUNK, SLOTS * MAXK), dtype=np.float32)
        for s, b in enumerate(perm):
            e = int(eff[b])
            for k, rows in enumerate(slot_rows[s]):
                lo = k * PCHUNK
                n_valid = min(max(e - lo, 0), PCHUNK)
                if n_valid > 0:
                    maskt[:n_valid, s * MAXK + k] = 1.0 / e
        in_maps.append({"features": features[perm], "maskt": maskt})

    trace = os.environ.get("KERNEL_TRACE", "0") == "1"
    LAST_RESULTS = run_bass_kernel_spmd(
        nc,
        in_maps,
        core_ids=list(range(NCORES)),
        trace=trace,
        trace_cores=[0] if trace else None,
    )

    out = np.empty((B, D), dtype=np.float32)
    for c in range(NCORES):
        out[np.asarray(cores[c])] = LAST_RESULTS.results[c]["out"]
    return out
